# revision 2
# baseline (speedup 1.0000x reference)
"""Trainium2 Bass kernel for nn_AdaptiveLinearWithChannel.

Reference computation (per channel c of 64):
    bias_idx[c] = int(t[0, c, 0] * 31)
    out[c]      = x[c] @ W[model_idx[c]] + B[bias_idx[c]]
with x [64, 2048, 256] f32, W [64, 256, 256] f32, B [32, 256] f32.

Sharding: channels split 8-per-core across 8 NeuronCores (pure expert/data
parallel, no cross-device traffic). The per-channel weight gather
(W[model_idx]) and bias gather (B[bias_idx]) happen host-side while
sharding, per the sharding hint. x is passed to each core pre-transposed
to [c, d_in, n] so the TensorEngine contraction dim (d_in) lands on SBUF
partitions with large contiguous DMAs; the kernel computes
out_T[c] = Wg[c].T-free matmul producing [d_out, n], which the host
transposes back to [n, d_out].
"""

import os

import numpy as np

_N_CORES = 8
_C = 64           # channels
_N = 2048         # points per channel
_DIN = 256
_DOUT = 256
_NFRAMES = 32
_CLOC = _C // _N_CORES  # 8 channels per core

# matmul input dtype: "f32" (exact, 4 cyc/row), "f32r" (fast fp32, 1 cyc/row),
# "bf16" (inputs rounded to bf16, 1 cyc/row, half the x/W DMA traffic)
_VARIANT = os.environ.get("KERNEL_VARIANT", "f32r")

_compiled = {}
LAST_RESULTS = None  # test harness reads exec_time_ns off this


def _build(variant):
    import concourse.bacc as bacc
    import concourse.bass as bass
    import concourse.mybir as mybir
    import concourse.tile as tile

    f32 = mybir.dt.float32
    if variant == "bf16":
        in_dt = mybir.dt.bfloat16
        mm_dt = mybir.dt.bfloat16
    elif variant == "f32r":
        in_dt = f32
        mm_dt = mybir.dt.float32r
    else:
        in_dt = f32
        mm_dt = f32

    nc = bacc.Bacc("TRN2", target_bir_lowering=False, debug=False)

    xT = nc.declare_dram_parameter("xT", [_CLOC, _DIN, _N], in_dt, isOutput=False)
    Wg = nc.declare_dram_parameter("Wg", [_CLOC, _DIN, _DOUT], in_dt, isOutput=False)
    bgT = nc.declare_dram_parameter("bgT", [128, 2 * _CLOC], f32, isOutput=False)
    out = nc.declare_dram_parameter("out", [_CLOC, _DOUT, _N], f32, isOutput=True)

    NB = _N // 512  # 4 n-blocks of 512 per channel

    with tile.TileContext(nc) as tc:
        with (
            tc.tile_pool(name="xpool", bufs=3) as xpool,
            tc.tile_pool(name="wpool", bufs=3) as wpool,
            tc.tile_pool(name="bpool", bufs=1) as bpool,
            tc.tile_pool(name="opool", bufs=4) as opool,
            tc.tile_pool(name="psum", bufs=6, space=bass.MemorySpace.PSUM) as pspool,
        ):
            bias = bpool.tile([128, 2 * _CLOC], f32)
            nc.sync.dma_start(bias[:], bgT[:])

            for c in range(_CLOC):
                # x^T for this channel: [d_in=2*128, n] -> [128, 2, n]
                xt = xpool.tile([128, 2, _N], in_dt)
                nc.sync.dma_start(xt[:], xT[c].rearrange("(a p) n -> p a n", p=128))
                # weights: [d_in=2*128, d_out] -> [128, 2, d_out]
                wt = wpool.tile([128, 2, _DOUT], in_dt)
                nc.sync.dma_start(wt[:], Wg[c].rearrange("(a p) o -> p a o", p=128))

                for oc in range(2):
                    ot = opool.tile([128, _N], f32)
                    b_ap = bias[:, c * 2 + oc : c * 2 + oc + 1]
                    for nb in range(NB):
                        ps = pspool.tile([128, 512], f32)
                        lhs0 = wt[:, 0, oc * 128 : (oc + 1) * 128]
                        lhs1 = wt[:, 1, oc * 128 : (oc + 1) * 128]
                        rhs0 = xt[:, 0, nb * 512 : (nb + 1) * 512]
                        rhs1 = xt[:, 1, nb * 512 : (nb + 1) * 512]
                        if mm_dt != in_dt:
                            lhs0 = lhs0.bitcast(mm_dt)
                            lhs1 = lhs1.bitcast(mm_dt)
                            rhs0 = rhs0.bitcast(mm_dt)
                            rhs1 = rhs1.bitcast(mm_dt)
                        nc.tensor.matmul(ps[:], lhs0, rhs0, start=True, stop=False)
                        nc.tensor.matmul(ps[:], lhs1, rhs1, start=False, stop=True)
                        o_ap = ot[:, nb * 512 : (nb + 1) * 512]
                        # bias-add fused into the PSUM->SBUF copy; alternate
                        # engines so neither becomes the bottleneck
                        if nb % 2 == 0:
                            nc.vector.tensor_scalar_add(o_ap, ps[:], b_ap)
                        else:
                            nc.scalar.activation(
                                o_ap,
                                ps[:],
                                mybir.ActivationFunctionType.Identity,
                                bias=b_ap,
                            )
                    nc.sync.dma_start(out[c, oc * 128 : (oc + 1) * 128, :], ot[:])

    nc.compile()
    return nc


def kernel(x, t, model_idx, W, B):
    global LAST_RESULTS
    from concourse.bass_utils import run_bass_kernel_spmd

    x = np.asarray(x, dtype=np.float32)
    t = np.asarray(t, dtype=np.float32)
    model_idx = np.asarray(model_idx)
    W = np.asarray(W, dtype=np.float32)
    B = np.asarray(B, dtype=np.float32)

    # host-side routing (index tensors stay integer)
    bias_idx = (t[0, :, 0] * np.float32(_NFRAMES - 1)).astype(np.int32)
    Wg = W[model_idx]   # [64, 256, 256] gathered per-channel weights
    bg = B[bias_idx]    # [64, 256] gathered per-channel biases
    xT = np.ascontiguousarray(np.swapaxes(x, 1, 2))  # [64, 256, 2048]

    variant = _VARIANT
    if variant == "bf16":
        import ml_dtypes

        xT = xT.astype(ml_dtypes.bfloat16)
        Wg = Wg.astype(ml_dtypes.bfloat16)
    if variant not in _compiled:
        _compiled[variant] = _build(variant)
    nc = _compiled[variant]

    in_maps = []
    for k in range(_N_CORES):
        sl = slice(k * _CLOC, (k + 1) * _CLOC)
        # bias laid out for the device: bgT[p, c*2+oc] = bg[c, oc*128+p]
        bgT = np.ascontiguousarray(
            bg[sl].reshape(_CLOC, 2, 128).transpose(2, 0, 1).reshape(128, 2 * _CLOC)
        )
        in_maps.append(
            {
                "xT": np.ascontiguousarray(xT[sl]),
                "Wg": np.ascontiguousarray(Wg[sl]),
                "bgT": bgT,
            }
        )

    res = run_bass_kernel_spmd(nc, in_maps, core_ids=list(range(_N_CORES)))
    LAST_RESULTS = res

    out = np.empty((_C, _N, _DOUT), dtype=np.float32)
    for k in range(_N_CORES):
        outT = np.asarray(res.results[k]["out"])  # [8, 256, 2048]
        out[k * _CLOC : (k + 1) * _CLOC] = np.swapaxes(outT, 1, 2)
    return out


# revision 3
# speedup vs baseline: 1.3909x; 1.3909x over previous
"""Trainium2 Bass kernel for nn_AdaptiveLinearWithChannel.

Reference computation (per channel c of 64):
    bias_idx[c] = int(t[0, c, 0] * 31)
    out[c]      = x[c] @ W[model_idx[c]] + B[bias_idx[c]]
with x [64, 2048, 256] f32, W [64, 256, 256] f32, B [32, 256] f32.

Sharding: channels split 8-per-core across 8 NeuronCores (pure expert/data
parallel, no cross-device traffic). The per-channel weight gather
(W[model_idx]) and bias gather (B[bias_idx]) happen host-side while
sharding, per the sharding hint. x is passed to each core pre-transposed
to [c, d_in, n] so the TensorEngine contraction dim (d_in) lands on SBUF
partitions with large contiguous DMAs; the kernel computes
out_T[c] = Wg[c].T-free matmul producing [d_out, n], which the host
transposes back to [n, d_out].
"""

import os

import numpy as np

_N_CORES = 8
_C = 64           # channels
_N = 2048         # points per channel
_DIN = 256
_DOUT = 256
_NFRAMES = 32
_CLOC = _C // _N_CORES  # 8 channels per core

# matmul input dtype: "f32" (exact, 4 cyc/row), "f32r" (fast fp32, 1 cyc/row),
# "bf16" (inputs rounded to bf16, 1 cyc/row, half the x/W DMA traffic)
_VARIANT = os.environ.get("KERNEL_VARIANT", "f32r")

_compiled = {}
LAST_RESULTS = None  # test harness reads exec_time_ns off this


def _build(variant):
    import concourse.bacc as bacc
    import concourse.bass as bass
    import concourse.mybir as mybir
    import concourse.tile as tile

    f32 = mybir.dt.float32
    if variant == "bf16":
        in_dt = mybir.dt.bfloat16
        mm_dt = mybir.dt.bfloat16
    elif variant == "f32r":
        in_dt = mybir.dt.float32r
        mm_dt = mybir.dt.float32r
    else:
        in_dt = f32
        mm_dt = f32

    nc = bacc.Bacc("TRN2", target_bir_lowering=False, debug=False)

    xT = nc.declare_dram_parameter("xT", [_CLOC, _DIN, _N], in_dt, isOutput=False)
    Wg = nc.declare_dram_parameter("Wg", [_CLOC, _DIN, _DOUT], in_dt, isOutput=False)
    bgT = nc.declare_dram_parameter("bgT", [128, 2 * _CLOC], f32, isOutput=False)
    out = nc.declare_dram_parameter("out", [_CLOC, _DOUT, _N], f32, isOutput=True)

    NB = _N // 512  # 4 n-blocks of 512 per channel

    with tile.TileContext(nc) as tc:
        with (
            tc.tile_pool(name="xpool", bufs=3) as xpool,
            tc.tile_pool(name="wpool", bufs=3) as wpool,
            tc.tile_pool(name="bpool", bufs=1) as bpool,
            tc.tile_pool(name="opool", bufs=4) as opool,
            tc.tile_pool(name="psum", bufs=6, space=bass.MemorySpace.PSUM) as pspool,
        ):
            bias = bpool.tile([128, 2 * _CLOC], f32)
            nc.sync.dma_start(bias[:], bgT[:])

            for c in range(_CLOC):
                # x^T for this channel: [d_in=2*128, n] -> [128, 2, n]
                xt = xpool.tile([128, 2, _N], in_dt)
                nc.sync.dma_start(xt[:], xT[c].rearrange("(a p) n -> p a n", p=128))
                # weights: [d_in=2*128, d_out] -> [128, 2, d_out]
                wt = wpool.tile([128, 2, _DOUT], in_dt)
                nc.sync.dma_start(wt[:], Wg[c].rearrange("(a p) o -> p a o", p=128))

                for oc in range(2):
                    ot = opool.tile([128, _N], f32)
                    b_ap = bias[:, c * 2 + oc : c * 2 + oc + 1]
                    for nb in range(NB):
                        ps = pspool.tile([128, 512], f32)
                        lhs0 = wt[:, 0, oc * 128 : (oc + 1) * 128]
                        lhs1 = wt[:, 1, oc * 128 : (oc + 1) * 128]
                        rhs0 = xt[:, 0, nb * 512 : (nb + 1) * 512]
                        rhs1 = xt[:, 1, nb * 512 : (nb + 1) * 512]
                        if mm_dt != in_dt:
                            lhs0 = lhs0.bitcast(mm_dt)
                            lhs1 = lhs1.bitcast(mm_dt)
                            rhs0 = rhs0.bitcast(mm_dt)
                            rhs1 = rhs1.bitcast(mm_dt)
                        nc.tensor.matmul(ps[:], lhs0, rhs0, start=True, stop=False)
                        nc.tensor.matmul(ps[:], lhs1, rhs1, start=False, stop=True)
                        o_ap = ot[:, nb * 512 : (nb + 1) * 512]
                        # bias-add fused into the PSUM->SBUF copy; alternate
                        # engines so neither becomes the bottleneck
                        if nb % 2 == 0:
                            nc.vector.tensor_scalar_add(o_ap, ps[:], b_ap)
                        else:
                            nc.scalar.activation(
                                o_ap,
                                ps[:],
                                mybir.ActivationFunctionType.Identity,
                                bias=b_ap,
                            )
                    nc.sync.dma_start(out[c, oc * 128 : (oc + 1) * 128, :], ot[:])

    nc.compile()
    return nc


def kernel(x, t, model_idx, W, B):
    global LAST_RESULTS
    from concourse.bass_utils import run_bass_kernel_spmd

    x = np.asarray(x, dtype=np.float32)
    t = np.asarray(t, dtype=np.float32)
    model_idx = np.asarray(model_idx)
    W = np.asarray(W, dtype=np.float32)
    B = np.asarray(B, dtype=np.float32)

    # host-side routing (index tensors stay integer)
    bias_idx = (t[0, :, 0] * np.float32(_NFRAMES - 1)).astype(np.int32)
    Wg = W[model_idx]   # [64, 256, 256] gathered per-channel weights
    bg = B[bias_idx]    # [64, 256] gathered per-channel biases
    xT = np.ascontiguousarray(np.swapaxes(x, 1, 2))  # [64, 256, 2048]

    variant = _VARIANT
    if variant == "bf16":
        import ml_dtypes

        xT = xT.astype(ml_dtypes.bfloat16)
        Wg = Wg.astype(ml_dtypes.bfloat16)
    if variant not in _compiled:
        _compiled[variant] = _build(variant)
    nc = _compiled[variant]

    in_maps = []
    for k in range(_N_CORES):
        sl = slice(k * _CLOC, (k + 1) * _CLOC)
        # bias laid out for the device: bgT[p, c*2+oc] = bg[c, oc*128+p]
        bgT = np.ascontiguousarray(
            bg[sl].reshape(_CLOC, 2, 128).transpose(2, 0, 1).reshape(128, 2 * _CLOC)
        )
        in_maps.append(
            {
                "xT": np.ascontiguousarray(xT[sl]),
                "Wg": np.ascontiguousarray(Wg[sl]),
                "bgT": bgT,
            }
        )

    res = run_bass_kernel_spmd(nc, in_maps, core_ids=list(range(_N_CORES)))
    LAST_RESULTS = res

    out = np.empty((_C, _N, _DOUT), dtype=np.float32)
    for k in range(_N_CORES):
        outT = np.asarray(res.results[k]["out"])  # [8, 256, 2048]
        out[k * _CLOC : (k + 1) * _CLOC] = np.swapaxes(outT, 1, 2)
    return out


# revision 8
# speedup vs baseline: 1.8658x; 1.3414x over previous
"""Trainium2 Bass kernel for nn_AdaptiveLinearWithChannel.

Reference computation (per channel c of 64):
    bias_idx[c] = int(t[0, c, 0] * 31)
    out[c]      = x[c] @ W[model_idx[c]] + B[bias_idx[c]]
with x [64, 2048, 256] f32, W [64, 256, 256] f32, B [32, 256] f32.

Sharding: channels split 8-per-core across 8 NeuronCores (pure expert/data
parallel, no cross-device traffic). The per-channel weight gather
(W[model_idx]) and bias gather (B[bias_idx]) happen host-side while
sharding, per the sharding hint. x is passed to each core pre-transposed
to [c, d_in, n] so the TensorEngine contraction dim (d_in) lands on SBUF
partitions with large contiguous DMAs; the kernel computes
out_T[c] = Wg[c].T-free matmul producing [d_out, n], which the host
transposes back to [n, d_out].
"""

import os

import numpy as np

_N_CORES = 8
_C = 64           # channels
_N = 2048         # points per channel
_DIN = 256
_DOUT = 256
_NFRAMES = 32
_CLOC = _C // _N_CORES  # 8 channels per core

# matmul input dtype: "f32" (exact, 4 cyc/row), "f32r" (fast fp32, 1 cyc/row),
# "bf16" (inputs rounded to bf16, 1 cyc/row, half the x/W DMA traffic)
_VARIANT = os.environ.get("KERNEL_VARIANT", "f32r")

_compiled = {}
LAST_RESULTS = None  # test harness reads exec_time_ns off this


def _build(variant):
    import concourse.bacc as bacc
    import concourse.bass as bass
    import concourse.mybir as mybir
    import concourse.tile as tile

    f32 = mybir.dt.float32
    out_dt = f32
    if variant == "bf16":
        in_dt = mybir.dt.bfloat16
        mm_dt = mybir.dt.bfloat16
    elif variant == "bf16o":
        in_dt = mybir.dt.bfloat16
        mm_dt = mybir.dt.bfloat16
        out_dt = mybir.dt.bfloat16
    elif variant == "f32r":
        in_dt = mybir.dt.float32r
        mm_dt = mybir.dt.float32r
    else:
        in_dt = f32
        mm_dt = f32

    nc = bacc.Bacc("TRN2", target_bir_lowering=False, debug=False)

    xT = nc.declare_dram_parameter("xT", [_CLOC, _DIN, _N], in_dt, isOutput=False)
    Wg = nc.declare_dram_parameter("Wg", [_CLOC, _DIN, _DOUT], in_dt, isOutput=False)
    bgT = nc.declare_dram_parameter("bgT", [128, 2 * _CLOC], f32, isOutput=False)
    out = nc.declare_dram_parameter("out", [_CLOC, _DOUT, _N], out_dt, isOutput=True)

    NB = _N // 512  # 4 n-blocks of 512 per channel

    with tile.TileContext(nc) as tc:
        with (
            tc.tile_pool(name="xpool", bufs=3) as xpool,
            tc.tile_pool(name="wpool", bufs=3) as wpool,
            tc.tile_pool(name="bpool", bufs=1) as bpool,
            tc.tile_pool(name="opool", bufs=4) as opool,
            tc.tile_pool(name="psum", bufs=6, space=bass.MemorySpace.PSUM) as pspool,
        ):
            bias = bpool.tile([128, 2 * _CLOC], f32)
            nc.sync.dma_start(bias[:], bgT[:])

            for c in range(_CLOC):
                # x^T for this channel: [d_in=2*128, n] -> [128, 2, n]
                xt = xpool.tile([128, 2, _N], in_dt)
                nc.sync.dma_start(xt[:], xT[c].rearrange("(a p) n -> p a n", p=128))
                # weights: [d_in=2*128, d_out] -> [128, 2, d_out]
                wt = wpool.tile([128, 2, _DOUT], in_dt)
                nc.sync.dma_start(wt[:], Wg[c].rearrange("(a p) o -> p a o", p=128))

                ot = opool.tile([128, 2, _N], out_dt)
                for oc in range(2):
                    b_ap = bias[:, c * 2 + oc : c * 2 + oc + 1]
                    for nb in range(NB):
                        ps = pspool.tile([128, 512], f32)
                        lhs0 = wt[:, 0, oc * 128 : (oc + 1) * 128]
                        lhs1 = wt[:, 1, oc * 128 : (oc + 1) * 128]
                        rhs0 = xt[:, 0, nb * 512 : (nb + 1) * 512]
                        rhs1 = xt[:, 1, nb * 512 : (nb + 1) * 512]
                        if mm_dt != in_dt:
                            lhs0 = lhs0.bitcast(mm_dt)
                            lhs1 = lhs1.bitcast(mm_dt)
                            rhs0 = rhs0.bitcast(mm_dt)
                            rhs1 = rhs1.bitcast(mm_dt)
                        nc.tensor.matmul(ps[:], lhs0, rhs0, start=True, stop=False)
                        nc.tensor.matmul(ps[:], lhs1, rhs1, start=False, stop=True)
                        o_ap = ot[:, oc, nb * 512 : (nb + 1) * 512]
                        # bias-add fused into the PSUM->SBUF copy; alternate
                        # engines so neither becomes the bottleneck
                        if nb % 2 == 0:
                            nc.vector.tensor_scalar_add(o_ap, ps[:], b_ap)
                        else:
                            nc.scalar.activation(
                                o_ap,
                                ps[:],
                                mybir.ActivationFunctionType.Identity,
                                bias=b_ap,
                            )
                nc.sync.dma_start(
                    out[c].rearrange("(a p) n -> p a n", p=128), ot[:]
                )

    nc.compile()
    return nc


def kernel(x, t, model_idx, W, B):
    global LAST_RESULTS
    from concourse.bass_utils import run_bass_kernel_spmd

    x = np.asarray(x, dtype=np.float32)
    t = np.asarray(t, dtype=np.float32)
    model_idx = np.asarray(model_idx)
    W = np.asarray(W, dtype=np.float32)
    B = np.asarray(B, dtype=np.float32)

    # host-side routing (index tensors stay integer)
    bias_idx = (t[0, :, 0] * np.float32(_NFRAMES - 1)).astype(np.int32)
    Wg = W[model_idx]   # [64, 256, 256] gathered per-channel weights
    bg = B[bias_idx]    # [64, 256] gathered per-channel biases
    xT = np.ascontiguousarray(np.swapaxes(x, 1, 2))  # [64, 256, 2048]

    variant = _VARIANT
    if variant in ("bf16", "bf16o"):
        import ml_dtypes

        xT = xT.astype(ml_dtypes.bfloat16)
        Wg = Wg.astype(ml_dtypes.bfloat16)
    if variant not in _compiled:
        _compiled[variant] = _build(variant)
    nc = _compiled[variant]

    in_maps = []
    for k in range(_N_CORES):
        sl = slice(k * _CLOC, (k + 1) * _CLOC)
        # bias laid out for the device: bgT[p, c*2+oc] = bg[c, oc*128+p]
        bgT = np.ascontiguousarray(
            bg[sl].reshape(_CLOC, 2, 128).transpose(2, 0, 1).reshape(128, 2 * _CLOC)
        )
        in_maps.append(
            {
                "xT": np.ascontiguousarray(xT[sl]),
                "Wg": np.ascontiguousarray(Wg[sl]),
                "bgT": bgT,
            }
        )

    res = run_bass_kernel_spmd(nc, in_maps, core_ids=list(range(_N_CORES)))
    LAST_RESULTS = res

    out = np.empty((_C, _N, _DOUT), dtype=np.float32)
    for k in range(_N_CORES):
        outT = np.asarray(res.results[k]["out"]).astype(np.float32)  # [8, 256, 2048]
        out[k * _CLOC : (k + 1) * _CLOC] = np.swapaxes(outT, 1, 2)
    return out


# revision 9
# speedup vs baseline: 1.9005x; 1.0186x over previous
"""Trainium2 Bass kernel for nn_AdaptiveLinearWithChannel.

Reference computation (per channel c of 64):
    bias_idx[c] = int(t[0, c, 0] * 31)
    out[c]      = x[c] @ W[model_idx[c]] + B[bias_idx[c]]
with x [64, 2048, 256] f32, W [64, 256, 256] f32, B [32, 256] f32.

Sharding: channels split 8-per-core across 8 NeuronCores (pure expert/data
parallel, no cross-device traffic). The per-channel weight gather
(W[model_idx]) and bias gather (B[bias_idx]) happen host-side while
sharding, per the sharding hint. x is passed to each core pre-transposed
to [c, d_in, n] so the TensorEngine contraction dim (d_in) lands on SBUF
partitions with large contiguous DMAs; the kernel computes
out_T[c] = Wg[c].T-free matmul producing [d_out, n], which the host
transposes back to [n, d_out].
"""

import os

import numpy as np

_N_CORES = 8
_C = 64           # channels
_N = 2048         # points per channel
_DIN = 256
_DOUT = 256
_NFRAMES = 32
_CLOC = _C // _N_CORES  # 8 channels per core

# matmul input dtype: "f32" (exact, 4 cyc/row), "f32r" (fast fp32, 1 cyc/row),
# "bf16" (inputs rounded to bf16, 1 cyc/row, half the x/W DMA traffic)
_VARIANT = os.environ.get("KERNEL_VARIANT", "f32r")

_compiled = {}
LAST_RESULTS = None  # test harness reads exec_time_ns off this


def _build(variant):
    import concourse.bacc as bacc
    import concourse.bass as bass
    import concourse.mybir as mybir
    import concourse.tile as tile

    f32 = mybir.dt.float32
    out_dt = f32
    if variant == "bf16":
        in_dt = mybir.dt.bfloat16
        mm_dt = mybir.dt.bfloat16
    elif variant == "bf16o":
        in_dt = mybir.dt.bfloat16
        mm_dt = mybir.dt.bfloat16
        out_dt = mybir.dt.bfloat16
    elif variant == "f32r":
        in_dt = mybir.dt.float32r
        mm_dt = mybir.dt.float32r
    else:
        in_dt = f32
        mm_dt = f32

    nc = bacc.Bacc("TRN2", target_bir_lowering=False, debug=False)

    xT = nc.declare_dram_parameter("xT", [_CLOC, _DIN, _N], in_dt, isOutput=False)
    Wg = nc.declare_dram_parameter("Wg", [_CLOC, _DIN, _DOUT], in_dt, isOutput=False)
    bgT = nc.declare_dram_parameter("bgT", [128, 2 * _CLOC], f32, isOutput=False)
    out = nc.declare_dram_parameter("out", [_CLOC, _DOUT, _N], out_dt, isOutput=True)

    NB = _N // 512  # 4 n-blocks of 512 per channel

    with tile.TileContext(nc) as tc:
        with (
            tc.tile_pool(name="xpool", bufs=4) as xpool,
            tc.tile_pool(name="wpool", bufs=4) as wpool,
            tc.tile_pool(name="bpool", bufs=1) as bpool,
            tc.tile_pool(name="opool", bufs=4) as opool,
            tc.tile_pool(name="psum", bufs=8, space=bass.MemorySpace.PSUM) as pspool,
        ):
            bias = bpool.tile([128, 2 * _CLOC], f32)
            # gpsimd (SWDGE): keeps this 128-descriptor scatter off the
            # HWDGE ring so it doesn't delay the first big x DMA
            nc.gpsimd.dma_start(bias[:], bgT[:])

            for c in range(_CLOC):
                # x^T for this channel: [d_in=2*128, n] -> [128, 2, n]
                xt = xpool.tile([128, 2, _N], in_dt)
                nc.sync.dma_start(xt[:], xT[c].rearrange("(a p) n -> p a n", p=128))
                # weights: [d_in=2*128, d_out] -> [128, 2, d_out]
                wt = wpool.tile([128, 2, _DOUT], in_dt)
                nc.sync.dma_start(wt[:], Wg[c].rearrange("(a p) o -> p a o", p=128))

                ot = opool.tile([128, 2, _N], out_dt)
                for oc in range(2):
                    b_ap = bias[:, c * 2 + oc : c * 2 + oc + 1]
                    for nb in range(NB):
                        ps = pspool.tile([128, 512], f32)
                        lhs0 = wt[:, 0, oc * 128 : (oc + 1) * 128]
                        lhs1 = wt[:, 1, oc * 128 : (oc + 1) * 128]
                        rhs0 = xt[:, 0, nb * 512 : (nb + 1) * 512]
                        rhs1 = xt[:, 1, nb * 512 : (nb + 1) * 512]
                        if mm_dt != in_dt:
                            lhs0 = lhs0.bitcast(mm_dt)
                            lhs1 = lhs1.bitcast(mm_dt)
                            rhs0 = rhs0.bitcast(mm_dt)
                            rhs1 = rhs1.bitcast(mm_dt)
                        nc.tensor.matmul(ps[:], lhs0, rhs0, start=True, stop=False)
                        nc.tensor.matmul(ps[:], lhs1, rhs1, start=False, stop=True)
                        o_ap = ot[:, oc, nb * 512 : (nb + 1) * 512]
                        # bias-add fused into the PSUM->SBUF copy; alternate
                        # engines so neither becomes the bottleneck
                        if nb % 2 == 0:
                            nc.vector.tensor_scalar_add(o_ap, ps[:], b_ap)
                        else:
                            nc.scalar.activation(
                                o_ap,
                                ps[:],
                                mybir.ActivationFunctionType.Identity,
                                bias=b_ap,
                            )
                nc.sync.dma_start(
                    out[c].rearrange("(a p) n -> p a n", p=128), ot[:]
                )

    nc.compile()
    return nc


def kernel(x, t, model_idx, W, B):
    global LAST_RESULTS
    from concourse.bass_utils import run_bass_kernel_spmd

    x = np.asarray(x, dtype=np.float32)
    t = np.asarray(t, dtype=np.float32)
    model_idx = np.asarray(model_idx)
    W = np.asarray(W, dtype=np.float32)
    B = np.asarray(B, dtype=np.float32)

    # host-side routing (index tensors stay integer)
    bias_idx = (t[0, :, 0] * np.float32(_NFRAMES - 1)).astype(np.int32)
    Wg = W[model_idx]   # [64, 256, 256] gathered per-channel weights
    bg = B[bias_idx]    # [64, 256] gathered per-channel biases
    xT = np.ascontiguousarray(np.swapaxes(x, 1, 2))  # [64, 256, 2048]

    variant = _VARIANT
    if variant in ("bf16", "bf16o"):
        import ml_dtypes

        xT = xT.astype(ml_dtypes.bfloat16)
        Wg = Wg.astype(ml_dtypes.bfloat16)
    if variant not in _compiled:
        _compiled[variant] = _build(variant)
    nc = _compiled[variant]

    in_maps = []
    for k in range(_N_CORES):
        sl = slice(k * _CLOC, (k + 1) * _CLOC)
        # bias laid out for the device: bgT[p, c*2+oc] = bg[c, oc*128+p]
        bgT = np.ascontiguousarray(
            bg[sl].reshape(_CLOC, 2, 128).transpose(2, 0, 1).reshape(128, 2 * _CLOC)
        )
        in_maps.append(
            {
                "xT": np.ascontiguousarray(xT[sl]),
                "Wg": np.ascontiguousarray(Wg[sl]),
                "bgT": bgT,
            }
        )

    res = run_bass_kernel_spmd(nc, in_maps, core_ids=list(range(_N_CORES)))
    LAST_RESULTS = res

    out = np.empty((_C, _N, _DOUT), dtype=np.float32)
    for k in range(_N_CORES):
        outT = np.asarray(res.results[k]["out"]).astype(np.float32)  # [8, 256, 2048]
        out[k * _CLOC : (k + 1) * _CLOC] = np.swapaxes(outT, 1, 2)
    return out
